# revision 2
# baseline (speedup 1.0000x reference)
"""Distance-discriminator kernel for 8 Trainium2 cores (V3, bf16).

Math (reference): for x [N, D],
    sq[i,d] = sum_j (x[j,d]-x[i,d])^2 = N*(x[i,d]-mean_d)^2 + C_d,
    C_d = sum_j (x[j,d]-mean_d)^2,  mean_d = S_d/N
    out = log(sqrt(sq) + eps) @ W.T + b  with eps negligible (dist ~ sqrt(2N)).

Device mapping (columns d sharded 512/core, x shipped bf16 - rel err ~4e-3
against a 2e-2 gate; all accumulators fp32):
  1. S_d: column-quarter sums via gpsimd SWDGE accumulate-DMA (rides idle DMA
     engines), then a short bn_stats on the 1024-wide partial -> mean.
  2. u = (x - mean)^2: per-chunk on ACT (Square, accum_out -> C free) or DVE
     (tensor_scalar add + affine_mul_reduce v*v with accum -> C), balancing
     the two engines; bn_stats is DVE-only and Ln is ACT-only so the square
     is the only movable pass.
  3. logd2c = Ln(u * N*e^-C0 + C*e^-C0)  (centered by C0 so bf16 GEMM inputs
     carry fluctuation, not the ~8.9 mean; C0 folded into the host bias).
  4. out.T partial = (W/2)^T @ logd2c via bf16 matmuls into 8 PSUM banks,
     evacuated DVE/ACT, summed across cores on host (a device-side collective
     costs ~50us first-use on this stack), bias + C0 correction added on host.
"""

import numpy as np
import ml_dtypes

import concourse.bacc as bacc
import concourse.bass as bass
import concourse.tile as tile
from concourse import mybir
from concourse.bass_utils import run_bass_kernel_spmd

N = 4096          # rows
D = 4096          # feature columns
OUT = 64
NCORES = 8
DC = D // NCORES  # 512 columns per core
KCH = DC // 128   # 4 partition-chunks per core
C0 = 8.9          # ln(sq) centering constant; absorbed via host bias
EMC0 = float(np.exp(-C0))
ACT_SQ = (0, 3)   # chunks whose square runs on ACT (rest on DVE)

F32 = mybir.dt.float32
BF16 = mybir.dt.bfloat16
_cache: dict = {}


def _build():
    nc = bacc.Bacc(
        "TRN2",
        target_bir_lowering=False,
        debug=False,
        num_devices=NCORES,
    )
    xT = nc.dram_tensor("xT", [DC, N], BF16, kind="ExternalInput").ap()
    wT = nc.dram_tensor("wT", [128, KCH * OUT], BF16, kind="ExternalInput").ap()
    out = nc.dram_tensor("out", [OUT, N], F32, kind="ExternalOutput").ap()

    AL = mybir.AluOpType
    with tile.TileContext(nc) as tc:
        with (
            tc.tile_pool(name="wp", bufs=1) as wp,
            tc.tile_pool(name="xp", bufs=KCH) as xp,
            tc.tile_pool(name="zp", bufs=KCH) as zp,
            tc.tile_pool(name="st", bufs=KCH) as st,
            tc.tile_pool(name="up", bufs=2) as up,
            tc.tile_pool(name="lp", bufs=2) as lp,
            tc.tile_pool(name="pp", bufs=8, space="PSUM") as pp,
        ):
            # x: per chunk, column halves alternating between the two HWDGE
            # queues (sync / scalar) so both stream ~half the 4.2 MiB.
            xs = []
            for k in range(KCH):
                x_k = xp.tile([128, N], BF16, name=f"x_{k}", tag="x")
                nc.sync.dma_start(x_k[:, : N // 2], xT[k * 128 : (k + 1) * 128, : N // 2])
                nc.scalar.dma_start(x_k[:, N // 2 :], xT[k * 128 : (k + 1) * 128, N // 2 :])
                xs.append(x_k)
            w_all = wp.tile([128, KCH * OUT], BF16, name="w_all", tag="w_all")
            nc.scalar.dma_start(w_all[:], wT)
            # preload the Ln table set while ACT idles during the x stream
            # (the Square set loads implicitly at the first square)
            scr = wp.tile([128, 1], BF16, name="scr", tag="scr")
            nc.scalar.activation(
                scr[:], w_all[:, 0:1], mybir.ActivationFunctionType.Ln,
                bias=w_all[:, 1:2], scale=1.0,
            )

            out_sb = wp.tile([OUT, N], F32, name="out_sb", tag="out_sb")
            psums = [pp.tile([OUT, 512], F32, name=f"ps_{j}", tag="ps") for j in range(8)]

            for k in range(KCH):
                x_k = xs[k]
                # S-tree: z = q0+q1 (after half A), += q2+q3 (after half B)
                # on the gpsimd software-DGE queue - DMA engines do the adds.
                z_k = zp.tile([128, 1024], BF16, name=f"z_{k}", tag="z")
                nc.gpsimd.dma_start(z_k[:], x_k[:, 0:1024])
                for q in range(1, 4):
                    nc.gpsimd.dma_start(
                        z_k[:], x_k[:, q * 1024 : (q + 1) * 1024], accum_op=AL.add
                    )
                stats_k = st.tile([128, 2, 6], F32, name=f"stats_{k}", tag="stats")
                nc.vector.bn_stats(stats_k[:, 0, :], z_k[:, 0:512])
                nc.vector.bn_stats(stats_k[:, 1, :], z_k[:, 512:1024])
                mv_k = st.tile([128, 2], F32, name=f"mv_{k}", tag="mv")
                nc.vector.bn_aggr(mv_k[:], stats_k[:])
                # mean(z)*1024 = S ; negmean = -S/N
                nm_k = st.tile([128, 1], F32, name=f"nm_{k}", tag="nm")
                nc.vector.tensor_scalar_mul(nm_k[:], mv_k[:, 0:1], -1024.0 / N)

                u_k = up.tile([128, N], BF16, name=f"u_{k}", tag="u")
                C_k = st.tile([128, 1], F32, name=f"C_{k}", tag="C")
                if k in ACT_SQ:
                    nc.scalar.activation(
                        u_k[:], x_k[:], mybir.ActivationFunctionType.Square,
                        bias=nm_k[:], scale=1.0, accum_out=C_k[:],
                    )
                else:
                    v_k = up.tile([128, N], BF16, name=f"v_{k}", tag="v", bufs=1)
                    nc.vector.tensor_scalar(v_k[:], x_k[:], nm_k[:], None, op0=AL.add)
                    nc.vector.affine_mul_reduce(
                        u_k[:], C_k[:], v_k[:], v_k[:], scale=1.0, bias=0.0
                    )
                bC_k = st.tile([128, 1], F32, name=f"bC_{k}", tag="bC")
                nc.vector.tensor_scalar_mul(bC_k[:], C_k[:], EMC0)

                l_k = lp.tile([128, N], BF16, name=f"l_{k}", tag="l")
                npiece = 2 if k == KCH - 1 else 1
                wq = N // npiece
                for q in range(npiece):
                    nc.scalar.activation(
                        l_k[:, q * wq : (q + 1) * wq],
                        u_k[:, q * wq : (q + 1) * wq],
                        mybir.ActivationFunctionType.Ln,
                        bias=bC_k[:], scale=float(N) * EMC0,
                    )
                    for jj in range(4 * wq // 2048):
                        j = q * (4 * wq // 2048) + jj
                        nc.tensor.matmul(
                            psums[j][:],
                            lhsT=w_all[:, k * OUT : (k + 1) * OUT],
                            rhs=l_k[:, j * 512 : (j + 1) * 512],
                            start=(k == 0),
                            stop=(k == KCH - 1),
                        )

            # evacuate PSUM (no bias - host adds it); out DMA per pair of banks
            for j in range(8):
                if j < 6:
                    nc.vector.tensor_copy(out_sb[:, j * 512 : (j + 1) * 512], psums[j][:])
                else:
                    nc.scalar.copy(out_sb[:, j * 512 : (j + 1) * 512], psums[j][:])
                if j % 2 == 1:
                    nc.sync.dma_start(
                        out[:, (j - 1) * 512 : (j + 1) * 512],
                        out_sb[:, (j - 1) * 512 : (j + 1) * 512],
                    )

    nc.compile()
    return nc


def _prep_inputs(data, W, b):
    data = np.asarray(data, dtype=np.float32)
    W = np.asarray(W, dtype=np.float32)
    b = np.asarray(b, dtype=np.float32)
    xb = data.astype(ml_dtypes.bfloat16)               # [N, D] bf16
    w2T = (0.5 * W).T.astype(ml_dtypes.bfloat16)       # [D, OUT] bf16
    in_maps = []
    for c in range(NCORES):
        xT_c = np.ascontiguousarray(xb[:, c * DC : (c + 1) * DC].T)   # [DC, N]
        w_c = (
            w2T[c * DC : (c + 1) * DC, :]
            .reshape(KCH, 128, OUT)
            .transpose(1, 0, 2)
            .reshape(128, KCH * OUT)
        )
        in_maps.append({"xT": xT_c, "wT": np.ascontiguousarray(w_c)})
    host_bias = (b + C0 * (0.5 * W).sum(axis=1)).astype(np.float32)   # [OUT]
    return in_maps, host_bias


def _run(inputs, trace=False, **kwargs):
    if "nc" not in _cache:
        _cache["nc"] = _build()
    nc = _cache["nc"]
    in_maps, host_bias = _prep_inputs(inputs["data"], inputs["W"], inputs["b"])
    res = run_bass_kernel_spmd(
        nc, in_maps, core_ids=list(range(NCORES)), trace=trace, **kwargs
    )
    outT = np.sum([res.results[c]["out"] for c in range(NCORES)], axis=0, dtype=np.float32)
    return np.ascontiguousarray(outT.T + host_bias[None, :]), res


def kernel(data, W, b):
    out, _ = _run({"data": data, "W": W, "b": b})
    return out


# revision 3
# speedup vs baseline: 1.2512x; 1.2512x over previous
"""Distance-discriminator kernel for 8 Trainium2 cores (V4, bf16).

Math (reference): for x [N, D],
    sq[i,d] = sum_j (x[j,d]-x[i,d])^2 = N*(x[i,d]-mean_d)^2 + C_d,
    C_d = sum_j (x[j,d]-mean_d)^2,  mean_d = S_d/N
    out = log(sqrt(sq) + eps) @ W.T + b  with eps negligible (dist ~ sqrt(2N)).

Device mapping (columns d sharded 512/core, x shipped bf16 - rel err ~4e-4
measured against a 2e-2 gate; all accumulators fp32):
  1. mean_d: pairwise column-fold tree on DVE (tensor_tensor add at 2x bf16:
     4096->2048->1024) then two 512-wide bn_stats + aggregate. ~3.4us/chunk
     vs 5.8 for direct 8-segment bn_stats (bn is 1x-locked by hardware).
  2. u = (x - mean)^2: chunks 0,1 on ACT (Square, accum_out gives C free);
     chunks 2,3 on DVE (tensor_scalar add at 4x + affine_mul_reduce v*v,
     whose accum gives C). The square is the only pass that can move between
     the two engines (bn/AMR are DVE-only, Ln is ACT-only), so it balances
     them; late chunks sit on DVE so ACT's Ln stream is never input-starved.
  3. logd2c = Ln(u * N*e^-C0 + C*e^-C0)  (centered by C0 so bf16 GEMM inputs
     carry fluctuation, not the ~8.9 mean; C0 folded into the host bias).
  4. out.T partial = (W/2)^T @ logd2c via bf16 matmuls into 8 PSUM banks,
     evacuated DVE/ACT, summed across cores on host (a device-side collective
     costs ~50us first-use on this stack), bias + C0 correction added on host.
  x streams on all three DMA queues (sync / scalar HWDGE + gpsimd SWDGE).
"""

import numpy as np
import ml_dtypes

import concourse.bacc as bacc
import concourse.bass as bass
import concourse.tile as tile
from concourse import mybir
from concourse.bass_utils import run_bass_kernel_spmd

N = 4096          # rows
D = 4096          # feature columns
OUT = 64
NCORES = 8
DC = D // NCORES  # 512 columns per core
KCH = DC // 128   # 4 partition-chunks per core
C0 = 8.9          # ln(sq) centering constant; absorbed via host bias
EMC0 = float(np.exp(-C0))
ACT_SQ = (0, 1)   # chunks whose square runs on ACT (rest on DVE)

F32 = mybir.dt.float32
BF16 = mybir.dt.bfloat16
_cache: dict = {}


def _build():
    nc = bacc.Bacc(
        "TRN2",
        target_bir_lowering=False,
        debug=False,
        num_devices=NCORES,
    )
    xT = nc.dram_tensor("xT", [DC, N], BF16, kind="ExternalInput").ap()
    wT = nc.dram_tensor("wT", [128, KCH * OUT], BF16, kind="ExternalInput").ap()
    out = nc.dram_tensor("out", [OUT, N], F32, kind="ExternalOutput").ap()

    AL = mybir.AluOpType
    with tile.TileContext(nc) as tc:
        with (
            tc.tile_pool(name="wp", bufs=1) as wp,
            tc.tile_pool(name="xp", bufs=KCH) as xp,
            tc.tile_pool(name="zp", bufs=2) as zp,
            tc.tile_pool(name="st", bufs=KCH) as st,
            tc.tile_pool(name="up", bufs=2) as up,
            tc.tile_pool(name="lp", bufs=2) as lp,
            tc.tile_pool(name="pp", bufs=8, space="PSUM") as pp,
        ):
            # x halves ride the two HWDGE queues (sync/scalar); chunk 2 goes
            # whole on the gpsimd SWDGE queue, which otherwise sits idle.
            xs = []
            for k in range(KCH):
                x_k = xp.tile([128, N], BF16, name=f"x_{k}", tag="x")
                if k == 2:
                    nc.gpsimd.dma_start(x_k[:], xT[k * 128 : (k + 1) * 128, :])
                else:
                    nc.sync.dma_start(
                        x_k[:, : N // 2], xT[k * 128 : (k + 1) * 128, : N // 2]
                    )
                    nc.scalar.dma_start(
                        x_k[:, N // 2 :], xT[k * 128 : (k + 1) * 128, N // 2 :]
                    )
                xs.append(x_k)
            w_all = wp.tile([128, KCH * OUT], BF16, name="w_all", tag="w_all")
            nc.scalar.dma_start(w_all[:], wT)
            # preload the Ln table set while ACT idles during the x stream
            # (the Square set loads implicitly at the first square)
            scr = wp.tile([128, 1], BF16, name="scr", tag="scr")
            nc.scalar.activation(
                scr[:], w_all[:, 0:1], mybir.ActivationFunctionType.Ln,
                bias=w_all[:, 1:2], scale=1.0,
            )

            out_sb = wp.tile([OUT, N], F32, name="out_sb", tag="out_sb")
            psums = [pp.tile([OUT, 512], F32, name=f"ps_{j}", tag="ps") for j in range(8)]

            for k in range(KCH):
                x_k = xs[k]
                # column-fold tree: 4096 -> 2048 -> 1024 (bf16 adds at 2x),
                # then bn_stats on the 1024-wide partial sums -> mean/1024
                t1_k = zp.tile([128, N // 2], BF16, name=f"t1_{k}", tag="t1")
                nc.vector.tensor_tensor(
                    t1_k[:], x_k[:, : N // 2], x_k[:, N // 2 :], op=AL.add
                )
                t2_k = zp.tile([128, N // 4], BF16, name=f"t2_{k}", tag="t2")
                nc.vector.tensor_tensor(
                    t2_k[:], t1_k[:, : N // 4], t1_k[:, N // 4 :], op=AL.add
                )
                stats_k = st.tile([128, 2, 6], F32, name=f"stats_{k}", tag="stats")
                nc.vector.bn_stats(stats_k[:, 0, :], t2_k[:, 0:512])
                nc.vector.bn_stats(stats_k[:, 1, :], t2_k[:, 512:1024])
                mv_k = st.tile([128, 2], F32, name=f"mv_{k}", tag="mv")
                nc.vector.bn_aggr(mv_k[:], stats_k[:])
                # mean(t2) = S/1024 ; negmean = -S/N = -mean(t2)/4
                nm_k = st.tile([128, 1], F32, name=f"nm_{k}", tag="nm")
                nc.vector.tensor_scalar_mul(nm_k[:], mv_k[:, 0:1], -0.25)

                u_k = up.tile([128, N], BF16, name=f"u_{k}", tag="u")
                C_k = st.tile([128, 2], F32, name=f"C_{k}", tag="C")
                bC_k = st.tile([128, 1], F32, name=f"bC_{k}", tag="bC")
                if k in ACT_SQ:
                    nc.scalar.activation(
                        u_k[:], x_k[:], mybir.ActivationFunctionType.Square,
                        bias=nm_k[:], scale=1.0, accum_out=C_k[:, 0:1],
                    )
                    nc.vector.tensor_scalar_mul(bC_k[:], C_k[:, 0:1], EMC0)
                else:
                    v_k = up.tile([128, N], BF16, name=f"v_{k}", tag="v", bufs=1)
                    nc.vector.tensor_scalar(v_k[:], x_k[:], nm_k[:], None, op0=AL.add)
                    # halves so the tail chunk's Ln can start after half A
                    for h in range(2):
                        nc.vector.affine_mul_reduce(
                            u_k[:, h * (N // 2) : (h + 1) * (N // 2)],
                            C_k[:, h : h + 1],
                            v_k[:, h * (N // 2) : (h + 1) * (N // 2)],
                            v_k[:, h * (N // 2) : (h + 1) * (N // 2)],
                            scale=1.0, bias=0.0,
                        )
                    nc.vector.tensor_scalar(
                        bC_k[:], C_k[:, 0:1], C_k[:, 1:2], EMC0,
                        op0=AL.add, op1=AL.mult,
                    )

                l_k = lp.tile([128, N], BF16, name=f"l_{k}", tag="l")
                npiece = 2 if k == KCH - 1 else 1
                wq = N // npiece
                for q in range(npiece):
                    nc.scalar.activation(
                        l_k[:, q * wq : (q + 1) * wq],
                        u_k[:, q * wq : (q + 1) * wq],
                        mybir.ActivationFunctionType.Ln,
                        bias=bC_k[:], scale=float(N) * EMC0,
                    )
                    nb = wq // 512
                    for jj in range(nb):
                        j = q * nb + jj
                        nc.tensor.matmul(
                            psums[j][:],
                            lhsT=w_all[:, k * OUT : (k + 1) * OUT],
                            rhs=l_k[:, j * 512 : (j + 1) * 512],
                            start=(k == 0),
                            stop=(k == KCH - 1),
                        )

            # evacuate PSUM (no bias - host adds it); out DMA per pair of banks
            for j in range(8):
                if j < 5:
                    nc.vector.tensor_copy(out_sb[:, j * 512 : (j + 1) * 512], psums[j][:])
                else:
                    nc.scalar.copy(out_sb[:, j * 512 : (j + 1) * 512], psums[j][:])
                if j % 2 == 1:
                    nc.sync.dma_start(
                        out[:, (j - 1) * 512 : (j + 1) * 512],
                        out_sb[:, (j - 1) * 512 : (j + 1) * 512],
                    )

    nc.compile()
    return nc


def _prep_inputs(data, W, b):
    data = np.asarray(data, dtype=np.float32)
    W = np.asarray(W, dtype=np.float32)
    b = np.asarray(b, dtype=np.float32)
    xb = data.astype(ml_dtypes.bfloat16)               # [N, D] bf16
    w2T = (0.5 * W).T.astype(ml_dtypes.bfloat16)       # [D, OUT] bf16
    in_maps = []
    for c in range(NCORES):
        xT_c = np.ascontiguousarray(xb[:, c * DC : (c + 1) * DC].T)   # [DC, N]
        w_c = (
            w2T[c * DC : (c + 1) * DC, :]
            .reshape(KCH, 128, OUT)
            .transpose(1, 0, 2)
            .reshape(128, KCH * OUT)
        )
        in_maps.append({"xT": xT_c, "wT": np.ascontiguousarray(w_c)})
    host_bias = (b + C0 * (0.5 * W).sum(axis=1)).astype(np.float32)   # [OUT]
    return in_maps, host_bias


def _run(inputs, trace=False, **kwargs):
    if "nc" not in _cache:
        _cache["nc"] = _build()
    nc = _cache["nc"]
    in_maps, host_bias = _prep_inputs(inputs["data"], inputs["W"], inputs["b"])
    res = run_bass_kernel_spmd(
        nc, in_maps, core_ids=list(range(NCORES)), trace=trace, **kwargs
    )
    outT = np.sum([res.results[c]["out"] for c in range(NCORES)], axis=0, dtype=np.float32)
    return np.ascontiguousarray(outT.T + host_bias[None, :]), res


def kernel(data, W, b):
    out, _ = _run({"data": data, "W": W, "b": b})
    return out
